# revision 5
# baseline (speedup 1.0000x reference)
"""TRN2 Bass kernel for nn_BatchGraphEncoder (gnn_message_passing).

Strategy
--------
Data-parallel over batch B=8 (one scene per NeuronCore). The A x A edge state
is collapsed algebraically: es_new[i,j] = h[i]@W1 + h[j]@W2 + es[i,j]@W3 + b,
and the only consumption of es is its row-sum agg[i] = sum_j es_new[i,j], so
we track S[i] = sum_j es[i,j] with the recurrence
    S_new = A*(h@W1) + (sum_j h[j])@W2 + S@W3 + A*b_edge.
This removes all O(A^2) work. All layouts are feature-major [feat, col] with
column index c = t*64 + a ("t-major").

Precision: the recurrence is chaotic (perturbation gain ~2x/step), so fp16
storage is not enough, and fp32 matmuls on TRN2 are ~12x slower than fp16.
We use an fp16 hi/lo split (x = xh + xl) with 3-term products
    x@W ~= xh@Wh + xh@Wl + xl@Wh        (error ~2^-22, fp32-like)
running everything on fast fp16 matmuls with fp32 PSUM accumulation.

The LSTM gate nonlinearities use a tanh-only trick: sigmoid(x)=0.5tanh(0.5x)
+0.5, with the i/f/o weight columns pre-scaled by 0.5 on the host so one ACT
tanh covers all four gates; h is tracked as h~ = 2h with all h-consuming
weights pre-halved.

PSUM-resident gates: the per-step LSTM gate pre-activations G[t] are
accumulated DIRECTLY in PSUM banks 0-5 (layout col = g*768 + t*64 + a).
The G_pre matmuls write there with start=False after warm-up spin matmuls
zero each bank with start=True (setting has_written for the whole bank), and
the per-step h/ne matmuls accumulate on top. This removes the fp32 identity
matmul (~1.3us/step) and the ACT psum->SBUF G copies of the previous design.
Banks 6/7 hold (in time-disjoint slices) the coord psum, S/se init, the
nps/sps double-buffers and the pred psum.
"""
import numpy as np

import bass_rust
import concourse.bass as bass
import concourse.tile as tile
from concourse import mybir
from concourse.bass_utils import run_bass_kernel_spmd
from concourse.vector_clock import ScopedClock, VectorClock

B, A, T, H, E, O, TY, SC, AG = 8, 64, 12, 128, 128, 128, 8, 32, 16
NCOL = T * A           # 768 columns, c = t*64 + a
F16 = mybir.dt.float16
F32 = mybir.dt.float32
AF = mybir.ActivationFunctionType
ALU = mybir.AluOpType

# ---------------------------------------------------------------------------
# walrus rejects instructions with >4 sync-wait commands; TileContext's exit
# drain collects a wait for every live semaphore onto one Drain. Split them.
_N_PROCS = bass_rust.N_PROCS


def _patched_drain_and_barrier(self, tick_clock, wait_clock):
    gc = tick_clock.global_clock
    ticks = [gc.peek_next(p) - 1 for p in range(_N_PROCS)]
    nz = [p for p, t in enumerate(ticks) if t > 0]
    for i in range(0, len(nz), 1):
        chunk = set(nz[i:i + 1])
        part = VectorClock([ticks[p] if p in chunk else 0 for p in range(_N_PROCS)])
        d = self.nc.sync.drain()
        wait_clock.add_sem_waits(d.ins, ScopedClock({None: part}))
    self.nc.sync.drain()
    # single-shot NEFF: skip the exit barriers and semaphore clears (they cost
    # ~6us of EVSEM butterflies); each engine halts after its last instruction.
    assert self.sems is not None
    popped = self.nc._tile_sem_poison_stack.pop()
    assert popped is self._sem_poison


tile.TileContext._drain_and_barrier = _patched_drain_and_barrier


def _split_excess_waits(nc, limit=1):
    """walrus accepts only ~1 sync-wait command per TPB instruction; Tile can
    assign several. Move excess waits onto ENGINE_NOPs inserted just before
    the over-subscribed instruction (same engine => program order preserves
    the happens-before)."""
    eligible = {mybir.EngineType.PE, mybir.EngineType.DVE,
                mybir.EngineType.Activation, mybir.EngineType.Pool,
                mybir.EngineType.SP}
    n_split = 0
    for f in nc.m.functions:
        for bb in f.blocks:
            insts = bb.instructions
            i = 0
            while i < len(insts):
                inst = insts[i]
                si = inst.sync_info
                if (si is not None and len(si.on_wait) > limit
                        and inst.engine in eligible):
                    waits = list(si.on_wait)
                    extra, keep = waits[:-limit], waits[-limit:]
                    pos = i
                    for j in range(0, len(extra), limit):
                        chunk = extra[j:j + limit]
                        ev = mybir.InstEventSemaphore(
                            name=nc.get_next_instruction_name(), ins=[], outs=[])
                        ev.engine = inst.engine
                        ev.sync_info = bass_rust.SyncInfo(on_wait=chunk, on_update=[])
                        nc.register_instruction(ev)
                        insts.insert(pos, ev)
                        pos += 1
                        i += 1
                        n_split += 1
                    si.on_wait = keep
                i += 1
    return n_split


# ---------------------------------------------------------------------------
# host-side weight composition
def _s16(x):
    """fp16 hi/lo split of an fp64/fp32 array."""
    x = np.asarray(x, np.float64)
    h = x.astype(np.float16)
    l = (x - h.astype(np.float64)).astype(np.float16)
    return h, l


# wld dense blob: 36 blocks of 128 cols, all K=128, in arrival-priority order
_WLD_ORDER = ["W3eh", "W3el", "Wsseh", "Wssel", "Wcohh", "Wcohl", "W2ehh",
              "W2ehl",                                   # nps        (0:1024)
              "Whh2h", "Whh2l", "Wihneh", "Wihnel",      # gates 512w (1024:3072)
              "W3h", "W3l", "AW1hh", "AW1hl", "W2hh", "W2hl", "Wssh", "Wssl",
              "Wsh2h", "Wsh2l",                          # sps        (3072:4352)
              "Wpredhh", "Wpredhl"]                      # pred       (4352:4608)
_WLD_WIDE = {"Whh2h", "Whh2l", "Wihneh", "Wihnel"}       # 512-col blocks

# sm blob [42, 2880]: xc(1536) | stk(192) | win(256) | wcorr(512) | wsm(384)
_SM_XC, _SM_STK, _SM_WIN, _SM_WCORR, _SM_WSM = 0, 1536, 1728, 1984, 2496
_SM_COLS = 2880


def _wld_offsets():
    offs, c = {}, 0
    for n in _WLD_ORDER:
        w = 512 if n in _WLD_WIDE else 128
        offs[n] = c
        c += w
    assert c == 4608
    return offs


_WLD_OFF = _wld_offsets()

_VN = {"b_in": 0, "bg0": 1, "bg1": 2, "bg2": 3, "bg3": 4,
       "sbias": 5, "bself": 6, "s1c": 7, "b_pred": 8}


def _prep_weights(inp):
    """Compose all lhsT weight tiles (fp64 math, fp16 hi/lo)."""
    W_ih = np.asarray(inp["W_ih"], np.float64)
    W_hh = np.asarray(inp["W_hh"], np.float64)
    b_g = np.asarray(inp["b_ih"], np.float64) + np.asarray(inp["b_hh"], np.float64)
    # gate order (f,i,g,o): f's ACT output is sigmoid(zf) directly (so the
    # c update is a pure tensor-tensor multiply, legal on GPSIMD); i/o keep
    # the 0.5 tanh trick; g unscaled
    perm = np.concatenate([np.arange(H, 2 * H), np.arange(0, H),
                           np.arange(2 * H, 3 * H), np.arange(3 * H, 4 * H)])
    W_ih = W_ih[:, perm]
    W_hh = W_hh[:, perm]
    b_g = b_g[perm]
    cs = np.concatenate([np.full(2 * H, 0.5), np.ones(H), np.full(H, 0.5)])
    W_ih = W_ih * cs
    W_hh = W_hh * cs
    b_g = b_g * cs

    Wih_c = W_ih[0:H]
    Wih_ty = W_ih[H:2 * H]
    Wih_ne = W_ih[2 * H:3 * H]
    Wih_sc = W_ih[3 * H:4 * H]
    Wih_ag = W_ih[4 * H:5 * H]

    We = np.asarray(inp["W_edge"], np.float64)
    W1, W2, W3 = We[:H], We[H:2 * H], We[2 * H:]
    Ws = np.asarray(inp["W_self"], np.float64)
    Wsh, Wss, Wst = Ws[:H], Ws[H:H + E], Ws[H + E:]
    We2n = np.asarray(inp["W_e2n"], np.float64)
    Wnt = np.asarray(inp["W_node_type"], np.float64)
    Wsc = np.asarray(inp["W_scene"], np.float64)
    Wag = np.asarray(inp["W_agent"], np.float64)
    b_edge = np.asarray(inp["b_edge"], np.float64)
    b_self = np.asarray(inp["b_self"], np.float64)
    b_e2n = np.asarray(inp["b_e2n"], np.float64)

    tensors = {}

    # coord = relu(X6@W_in + b_in):  win [6, 256] = Winh | Winl
    Winh, Winl = _s16(np.asarray(inp["W_in"], np.float64))
    win = np.zeros((6, 256), np.float16)
    win[:, 0:128], win[:, 128:256] = Winh, Winl

    # G_pre coord chunk (K=128, 3-term): wcg [128, 1024] = Wcgh | Wcgl
    Wcgh, Wcgl = _s16(Wih_c)
    wcg = np.zeros((128, 1024), np.float16)
    wcg[:, 0:512], wcg[:, 512:1024] = Wcgh, Wcgl

    # G_pre merged chunk rows: [agent(16); ty(8); sc(32); rel(1); ones(1)]
    # (agent first: DVE writes need quadrant-aligned partition starts).
    # The ones row carries the full gate bias so the G regions need no
    # separate bias add.
    ne_cst = A * b_edge @ We2n + b_self @ We2n + b_e2n
    bg_full = (b_g + np.asarray(inp["b_node_type"], np.float64) @ Wih_ty
               + np.asarray(inp["b_scene"], np.float64) @ Wih_sc + ne_cst @ Wih_ne)
    Wmg = np.vstack([Wag @ Wih_ag,
                     Wnt @ Wih_ty + Wst @ We2n @ Wih_ne,
                     Wsc @ Wih_sc,
                     (np.asarray(inp["b_agent"], np.float64) @ Wih_ag)[None],
                     bg_full[None]])
    Wmgh_, Wmgl_ = _s16(Wmg)                      # [58,512]
    wmg = np.zeros((122, 1024), np.float16)       # WmgA | Wmgl(58 rows)
    wmg[0:58, 0:512] = Wmgh_
    wmg[64:122, 0:512] = Wmgh_
    wmg[0:58, 512:1024] = Wmgl_

    # t=0 correction: remove ne-coupled ty composite + ne const biases
    Cty = -(Wst @ We2n @ Wih_ne)
    Db0 = -(ne_cst @ Wih_ne)
    Ctyh, Ctyl = _s16(Cty)
    Dbh, Dbl = _s16(Db0[None])
    wcorr = np.vstack([Ctyh, Ctyl, Ctyh, Dbh, Dbl]).astype(np.float16)  # [33? no 26.. ] K=26
    # rows: ty(8)h, ty(8)l?? keep the baseline stacking: rhs corrstk has rows
    # [tyh; tyh; tyl; ones; ones] (26) so lhsT = [Ctyh; Ctyl; Ctyh; Dbh; Dbl] (26)
    assert wcorr.shape == (26, 512)

    # recurrence composites (h~=2h folding: h-consumers x0.5)
    comps = {
        "W3e": W3 @ We2n, "Wsse": Wss @ We2n,
        "Wcoh": ((A * W1 + Wsh) * 0.5) @ We2n, "W2eh": (W2 * 0.5) @ We2n,
        "W3": W3, "AW1h": A * W1 * 0.5, "W2h": W2 * 0.5, "Wss": Wss,
        "Wsh2": Wsh * 0.5, "Wihne": Wih_ne, "Whh2": W_hh * 0.5,
        "Wpredh": np.asarray(inp["W_pred"], np.float64) * 0.5,
    }
    wld = np.zeros((128, 4608), np.float16)
    for base, arr in comps.items():
        hh, ll = _s16(arr)
        for suf, a_ in (("h", hh), ("l", ll)):
            off = _WLD_OFF[base + suf]
            wld[:, off:off + a_.shape[1]] = a_

    # S init: S_1 = f0ty@(A*Wp@W3) + 1*(sum_j f0ty_j)@(Wq@W3) + const
    Wei = np.asarray(inp["W_edge_in"], np.float64)
    Wet = np.asarray(inp["W_edge_type"], np.float64)
    Wp = np.vstack([Wei[0:6], Wei[12:20] + Wet[0:8]])
    Wq = np.vstack([Wei[6:12], Wei[20:28] + Wet[8:16]])
    Wp3 = A * (Wp @ W3)
    Wq3 = Wq @ W3

    def stk6(w):                                  # [14,128] -> rhs s1stk K=42
        wh, wl = _s16(w)
        return np.vstack([wh[0:6], wl[0:6], wh[0:6],
                          wh[6:14], wl[6:14], wh[6:14]])

    Wsth, Wstl = _s16(Wst)
    wsm = np.zeros((42, 384), np.float16)         # Wp3s | Wq3s | Wstk(24r)
    wsm[:, 0:128] = stk6(Wp3)
    wsm[:, 128:256] = stk6(Wq3)
    wsm[0:24, 256:384] = np.vstack([Wsth, Wstl, Wsth])

    cst = (np.asarray(inp["b_edge_in"], np.float64)
           + np.asarray(inp["b_edge_type"], np.float64))
    vec32 = np.zeros((128, 9), np.float32)
    vec32[:, 0] = np.asarray(inp["b_in"], np.float64)
    for g in range(4):
        vec32[:, 1 + g] = bg_full[g * H:(g + 1) * H]
    vec32[:, 5] = A * b_edge
    vec32[:, 6] = b_self
    vec32[:, 7] = A * (cst @ W3) + A * b_edge
    vec32[:, 8] = np.asarray(inp["b_pred"], np.float64)

    tensors.update(win=win, wcg=wcg, wmg=wmg, wcorr=wcorr, wsm=wsm,
                   wld=wld, vec32=vec32)
    return tensors


def _prep_core_inputs(inp, b):
    """Per-core marshaled inputs:
    sm  [42, 2496]: xc [6,1536] | stk [42,192] | win | wcorr | wsm (weights
                    are folded into sm per core to save DMA issue slots)
    mst [122, 768]: msth rows 0:58, mstl rows 64:122
    xag [48, 768]:  only for the non-all-ones relevant_agents path
    """
    norm = np.asarray(inp["normalized_trajectories"][b], np.float64)
    traj = np.asarray(inp["trajectories"][b], np.float64)
    ag = np.asarray(inp["agent_data"][b], np.float64)
    ty = np.asarray(inp["agent_type"][b], np.float64)        # [A,TY]
    sc = np.asarray(inp["scene_data"][b], np.float64)        # [T,SC]
    rel = np.asarray(inp["relevant_agents"][b], np.float64)  # [A]

    Xc = np.concatenate([norm, traj], -1).transpose(2, 1, 0).reshape(6, NCOL)
    Xag = ag.transpose(2, 1, 0).reshape(16, NCOL)
    Xch, Xcl = _s16(Xc)
    Xagh, Xagl = _s16(Xag)
    tyh, tyl = _s16(ty.T)                                    # [8,A]
    sch, scl = _s16(sc.T)                                    # [32,T]

    tyb_h = np.repeat(tyh[:, None, :], T, 1).reshape(8, NCOL)
    tyb_l = np.repeat(tyl[:, None, :], T, 1).reshape(8, NCOL)
    scb_h = np.repeat(sch[:, :, None], A, 2).reshape(32, NCOL)
    scb_l = np.repeat(scl[:, :, None], A, 2).reshape(32, NCOL)
    relrow = np.tile(rel, T)[None]                           # [1,768]
    relh, _ = _s16(relrow)
    z16 = np.zeros((16, NCOL), np.float16)
    z1 = np.zeros((1, NCOL), np.float16)
    one1 = np.ones((1, NCOL), np.float16)
    msth0 = np.vstack([z16, tyb_h, scb_h, relh, one1]).astype(np.float16)  # [58,768]
    mstl0 = np.vstack([z16, tyb_l, scb_l, z1, z1]).astype(np.float16)

    ones = np.ones((1, A), np.float16)
    corrstk = np.vstack([tyh, tyh, tyl, ones, ones])         # [26,A]
    tystk = np.vstack([tyh, tyh, tyl])                       # [24,A]
    f0h, f0l = Xch[0:6, 0:A], Xcl[0:6, 0:A]
    s1stk = np.vstack([f0h, f0h, f0l, tyh, tyh, tyl])        # [42,A]

    ones_fast = bool(np.all(rel == 1.0))
    mst = np.zeros((122, NCOL), np.float16)
    if ones_fast:
        # mask multiply is an exact identity: agent rows go straight in
        msth0[0:16] = Xagh
        mstl0[0:16] = Xagl
    mst[0:58] = msth0
    mst[64:122] = mstl0

    sm = np.zeros((42, _SM_COLS), np.float16)
    sm[0:6, _SM_XC:_SM_XC + 768] = Xch
    sm[0:6, _SM_XC + 768:_SM_XC + 1536] = Xcl
    sm[0:42, _SM_STK:_SM_STK + 64] = s1stk
    sm[0:24, _SM_STK + 64:_SM_STK + 128] = tystk
    sm[0:26, _SM_STK + 128:_SM_STK + 192] = corrstk
    xag = np.zeros((48, NCOL), np.float16)
    xag[0:16] = Xagh
    xag[16:32] = Xagl
    xag[32:48] = np.repeat(relrow, 16, 0).astype(np.float16)
    return {"sm": sm, "mst": mst, "xag": xag, "ones_fast": ones_fast}


# ---------------------------------------------------------------------------
def _build(nc, ones_fast=True):
    """Emit the single-core program."""
    sm_ap = nc.dram_tensor("sm", [42, _SM_COLS], F16, kind="ExternalInput").ap()
    mst_ap = nc.dram_tensor("mst", [122, NCOL], F16, kind="ExternalInput").ap()
    vec_ap = nc.dram_tensor("vec32", [128, 9], F32, kind="ExternalInput").ap()
    wcg_ap = nc.dram_tensor("wcg", [128, 1024], F16, kind="ExternalInput").ap()
    wmg_ap = nc.dram_tensor("wmg", [122, 1024], F16, kind="ExternalInput").ap()
    wld_ap = nc.dram_tensor("wld", [128, 4608], F16, kind="ExternalInput").ap()
    out_ap = nc.dram_tensor("out", [128, NCOL], F32, kind="ExternalOutput").ap()
    if not ones_fast:
        xag_ap = nc.dram_tensor("xag", [48, NCOL], F16, kind="ExternalInput").ap()

    dbg = {}

    with tile.TileContext(nc) as tc:
        with (
            tc.tile_pool(name="gp", bufs=1, space="PSUM") as gpool,
            tc.tile_pool(name="bp", bufs=1, space="PSUM") as bpool,
            tc.tile_pool(name="per", bufs=1) as per,
            tc.tile_pool(name="stp", bufs=3) as stp,
        ):
            # ---- PSUM arena: banks 0-5 = gates G (col g*768 + t*64 + a);
            # banks 6/7 = coord / s1 / nps / sps / pred scratch.
            Gps = gpool.tile([128, 3072], F32, name="Gps")
            b6 = bpool.tile([128, 512], F32, name="b6")
            b7 = bpool.tile([128, 512], F32, name="b7")
            G3 = Gps[:].rearrange("p (g r) -> p g r", g=4)   # [128, 4, 768]

            # ---- persistent SBUF tiles + DMAs (priority order)
            sm = per.tile([42, _SM_COLS], F16)
            mst = per.tile([122, NCOL], F16)
            vec = per.tile([128, 9], F32)
            wcg = per.tile([128, 1024], F16)
            wmg = per.tile([122, 1024], F16)
            wld = per.tile([128, 4608], F16)
            nc.sync.dma_start(sm[:], sm_ap)
            nc.sync.dma_start(mst[:], mst_ap)
            nc.sync.dma_start(vec[:], vec_ap)
            nc.sync.dma_start(wcg[:], wcg_ap)
            nc.sync.dma_start(wmg[:], wmg_ap)
            nc.sync.dma_start(wld[:, 0:1024], wld_ap[:, 0:1024])
            nc.sync.dma_start(wld[:, 1024:3072], wld_ap[:, 1024:3072])
            nc.sync.dma_start(wld[:, 3072:4608], wld_ap[:, 3072:4608])
            if not ones_fast:
                xag = per.tile([48, NCOL], F16)
                nc.sync.dma_start(xag[:], xag_ap)

            Xch = sm[0:6, _SM_XC:_SM_XC + 768]
            Xcl = sm[0:6, _SM_XC + 768:_SM_XC + 1536]
            s1s = sm[0:42, _SM_STK:_SM_STK + 64]
            tys = sm[0:24, _SM_STK + 64:_SM_STK + 128]
            corrs = sm[0:26, _SM_STK + 128:_SM_STK + 192]
            msth = mst[0:58, :]

            def W(name):
                if name in ("Winh", "Winl"):
                    off = 0 if name == "Winh" else 128
                    return sm[0:6, _SM_WIN + off:_SM_WIN + off + 128]
                if name == "Wp3s":
                    return sm[0:42, _SM_WSM:_SM_WSM + 128]
                if name == "Wq3s":
                    return sm[0:42, _SM_WSM + 128:_SM_WSM + 256]
                if name == "Wstk":
                    return sm[0:24, _SM_WSM + 256:_SM_WSM + 384]
                off = _WLD_OFF[name]
                return wld[:, off:off + 128]

            def Wg(name, g):
                if name == "Wcgh":
                    return wcg[:, g * 128:(g + 1) * 128]
                if name == "Wcgl":
                    return wcg[:, 512 + g * 128:512 + (g + 1) * 128]
                if name == "WmgA":
                    return wmg[0:122, g * 128:(g + 1) * 128]
                if name == "Wmgl":
                    return wmg[0:58, 512 + g * 128:512 + (g + 1) * 128]
                if name == "Wcorr":
                    return sm[0:26, _SM_WCORR + g * 128:_SM_WCORR + (g + 1) * 128]
                off = _WLD_OFF[name]
                return wld[:, off + g * 128:off + (g + 1) * 128]

            def V(name):
                i = _VN[name]
                return vec[:, i:i + 1]

            coh = per.tile([128, NCOL], F16)
            col = per.tile([128, NCOL], F16)
            co32 = per.tile([128, NCOL], F32)
            hah = per.tile([128, NCOL], F16)         # h~ hi, col t*64+a
            hal = per.tile([128, NCOL], F16)
            out32 = per.tile([128, NCOL], F32)
            dbg.update(Gps=Gps, coh=coh, col=col, hah=hah, hal=hal)

            # warm the ACT tanh/relu table set during the DMAs
            warm = per.tile([1, 1], F32)
            nc.vector.memset(warm[:], 0.0)
            warm2 = per.tile([1, 1], F32)
            nc.scalar.activation(warm2[:], warm[:], AF.Tanh)

            nc.vector.memset(hal[:, 9 * 64:NCOL], 0.0)

            # spin the PE on a zeroed scratch tile during the DMA wait so the
            # HAM clock-gate reaches 8/8 before the real matmuls start. The
            # spins target the 6 G banks with start=True: each write zeroes
            # the bank AND sets has_written for all 512 cols, so every later
            # G matmul can accumulate with start=False.
            scr = per.tile([128, 512], F16)
            nc.vector.memset(scr[:], 0.0)
            for k in range(12):
                bk = k % 6
                nc.tensor.matmul(Gps[:, bk * 512:(bk + 1) * 512],
                                 scr[0:128, 0:128], scr[:],
                                 start=True, stop=(k >= 6),
                                 skip_group_check=True)

            if not ones_fast:
                # mask agent rows: m = (Xagh+Xagl) * rel (fp32), hi/lo into mst
                ms32 = per.tile([16, NCOL], F32)
                nc.vector.tensor_add(ms32[:], xag[0:16, :], xag[16:32, :])
                m32 = per.tile([16, NCOL], F32)
                nc.vector.tensor_mul(m32[:], ms32[:], xag[32:48, :])
                nc.vector.tensor_copy(mst[0:16, :], m32[:])
                nc.vector.tensor_sub(mst[64:80, :], m32[:], mst[0:16, :])

            # ---- coord = relu(X6@W_in + b_in), hi/lo
            for hf in range(2):
                s = slice(hf * 384, hf * 384 + 384)
                cps = (b6 if hf == 0 else b7)[:, 0:384]
                nc.tensor.matmul(cps, W("Winh"), Xch[:, s], start=True, stop=False)
                nc.tensor.matmul(cps, W("Winl"), Xch[:, s], start=False, stop=False)
                nc.tensor.matmul(cps, W("Winh"), Xcl[:, s], start=False, stop=True)
                nc.scalar.activation(co32[:, s], cps, AF.Relu, bias=V("b_in"))
            nc.vector.tensor_copy(coh[:], co32[:])
            nc.vector.tensor_sub(col[:], co32[:], coh[:])

            # ---- G_pre into PSUM gate banks: windows within one bank per gate
            def emit_G(g, t0, t1):
                c0, c1 = t0 * 64, t1 * 64
                dst = Gps[:, g * 768 + c0:g * 768 + c1]
                s = slice(c0, c1)
                nc.tensor.matmul(dst, Wg("Wcgh", g), coh[:, s], start=False,
                                 stop=False, skip_group_check=True)
                nc.tensor.matmul(dst, Wg("Wcgl", g), coh[:, s], start=False,
                                 stop=False, skip_group_check=True)
                nc.tensor.matmul(dst, Wg("Wcgh", g), col[:, s], start=False,
                                 stop=False, skip_group_check=True)
                nc.tensor.matmul(dst, Wg("WmgA", g), mst[:, s], start=False,
                                 stop=False, skip_group_check=True)
                nc.tensor.matmul(dst, Wg("Wmgl", g), msth[:, s], start=False,
                                 stop=(t0 > 0), skip_group_check=True)
                if t0 == 0:
                    # t=0 correction accumulates onto the t0 chunk
                    nc.tensor.matmul(Gps[:, g * 768:g * 768 + 64],
                                     Wg("Wcorr", g), corrs[:],
                                     start=False, stop=True,
                                     skip_group_check=True)

            for g in range(4):
                emit_G(g, 0, 1)         # t=0 slices: unblock the t0 LSTM

            # ---- S1 / se1 init (bank6 cols 384:512, bank7 cols 384:448)
            s1sps = b6[:, 384:512]
            nc.tensor.matmul(s1sps[:, 0:64], W("Wp3s"), s1s[:], start=True, stop=True)
            qps = b7[:, 384:448]
            nc.tensor.matmul(qps, W("Wq3s"), s1s[:], start=True, stop=True)
            qsum = stp.tile([128, 1], F32, name="qsum", tag="hred")
            nc.vector.tensor_reduce(qsum[:], qps, mybir.AxisListType.X, ALU.add)
            qsum2 = stp.tile([128, 1], F32, name="qsum2", tag="hv2")
            nc.vector.tensor_scalar_add(qsum2[:], qsum[:], V("s1c"))
            S_h = stp.tile([128, 64], F16, name="S_h0", tag="S_h")
            S_l = stp.tile([128, 64], F16, name="S_l0", tag="S_l")
            nc.vector.tensor_scalar_add(S_h[:], s1sps[:, 0:64], qsum2[:])
            nc.vector.scalar_tensor_tensor(S_l[:], s1sps[:, 0:64], qsum2[:], S_h[:],
                                           ALU.add, ALU.subtract)
            nc.tensor.matmul(s1sps[:, 64:128], W("Wstk"), tys[:],
                             start=False, stop=True, skip_group_check=True)
            se_h = stp.tile([128, 64], F16, name="se_h0", tag="se_h")
            se_l = stp.tile([128, 64], F16, name="se_l0", tag="se_l")
            nc.vector.tensor_scalar_add(se_h[:], s1sps[:, 64:128], V("bself"))
            nc.vector.scalar_tensor_tensor(se_l[:], s1sps[:, 64:128], V("bself"),
                                           se_h[:], ALU.add, ALU.subtract)
            dbg.update(S_h0=S_h, S_l0=S_l, se_h0=se_h)

            # ---- t=0 LSTM (h0 = c0 = 0); gate chunks (f,i,g,o)
            def lstm_tail(t, Tt, ctil_prev):
                # Tt layout: [0:64]=sigmoid(zf), [64:128]=tanh(zi/2),
                #            [128:192]=tanh(zg), [192:256]=tanh(zo/2)
                ctil = stp.tile([128, 64], F32, name=f"ctil{t}", tag="ctil")
                if ctil_prev is None:
                    nc.vector.scalar_tensor_tensor(ctil[:], Tt[:, 64:128], 1.0,
                                                   Tt[:, 128:192], ALU.add, ALU.mult)
                else:
                    # u = sigmoid(zf) * ctil_prev on GPSIMD (pure TT),
                    # in parallel with v on DVE
                    u = stp.tile([128, 64], F32, name=f"u{t}", tag="u")
                    nc.gpsimd.tensor_mul(u[:], Tt[:, 0:64], ctil_prev[:])
                    v = stp.tile([128, 64], F32, name=f"v{t}", tag="v")
                    nc.vector.scalar_tensor_tensor(v[:], Tt[:, 64:128], 1.0,
                                                   Tt[:, 128:192], ALU.add, ALU.mult)
                    nc.vector.tensor_add(ctil[:], u[:], v[:])
                tc32 = stp.tile([128, 64], F32, name=f"tc{t}", tag="tc")
                nc.scalar.activation(tc32[:], ctil[:], AF.Tanh, scale=0.5)
                h32 = stp.tile([128, 64], F32, name=f"h32_{t}", tag="h32")
                hred = stp.tile([128, 1], F32, name=f"hred{t}", tag="hred")
                nc.vector.scalar_tensor_tensor(h32[:], Tt[:, 192:256], 1.0,
                                               tc32[:], ALU.add, ALU.mult,
                                               accum_out=hred[:])
                hsl = slice(t * 64, t * 64 + 64)
                nc.vector.tensor_copy(hah[:, hsl], h32[:])
                hrh = stp.tile([128, 1], F16, name=f"hrh{t}", tag="hrh")
                nc.vector.tensor_copy(hrh[:], hred[:])
                hrl = None
                if t <= 8:
                    nc.vector.tensor_sub(hal[:, hsl], h32[:], hah[:, hsl])
                    hrl = stp.tile([128, 1], F16, name=f"hrl{t}", tag="hrl")
                    nc.vector.tensor_sub(hrl[:], hred[:], hrh[:])
                return ctil, hrh, hrl

            T0 = stp.tile([128, 256], F32, name="T0", tag="T")
            nc.scalar.activation(T0[:, 64:256], G3[:, 1:4, 0:64], AF.Tanh)
            ctil, hrh, hrl = lstm_tail(0, T0, None)
            for g in range(4):
                emit_G(g, 1, 4)         # t=1..3 slices, overlap the t0 tail

            def emit_pred(c0, c1):
                s = slice(c0, c1)
                pps = b7[:, 130:130 + (c1 - c0)]
                nc.tensor.matmul(pps, W("Wpredhh"), hah[:, s], start=True,
                                 stop=False, skip_group_check=True)
                nc.tensor.matmul(pps, W("Wpredhl"), hah[:, s], start=False,
                                 stop=False, skip_group_check=True)
                nc.tensor.matmul(pps, W("Wpredhh"), hal[:, s], start=False,
                                 stop=True, skip_group_check=True)
                nc.scalar.activation(out32[:, s], pps, AF.Relu, bias=V("b_pred"))
                nc.sync.dma_start(out_ap[:, s], out32[:, s])

            # ---- recurrence steps t=1..11
            for t in range(1, T):
                hp = slice((t - 1) * 64, t * 64)
                full = t <= 8   # steps 9-11: 2-term fp16 (error amplified <= 4x)

                # ne psum [128,65] in bank7 (double-buffered): S/se terms
                # first (ready early), then h, then the hred broadcast terms
                nb = b7[:, 65 * ((t - 1) % 2):65 * ((t - 1) % 2) + 65]
                nc.tensor.matmul(nb[:, 0:64], W("W3eh"), S_h[:], start=True, stop=False)
                nc.tensor.matmul(nb[:, 0:64], W("W3el"), S_h[:], start=False, stop=False)
                if full:
                    nc.tensor.matmul(nb[:, 0:64], W("W3eh"), S_l[:], start=False, stop=False)
                nc.tensor.matmul(nb[:, 0:64], W("Wsseh"), se_h[:], start=False, stop=False)
                nc.tensor.matmul(nb[:, 0:64], W("Wssel"), se_h[:], start=False, stop=False)
                if full:
                    nc.tensor.matmul(nb[:, 0:64], W("Wsseh"), se_l[:], start=False, stop=False)
                nc.tensor.matmul(nb[:, 0:64], W("Wcohh"), hah[:, hp], start=False, stop=False)
                nc.tensor.matmul(nb[:, 0:64], W("Wcohl"), hah[:, hp], start=False,
                                 stop=not full)
                if full:
                    nc.tensor.matmul(nb[:, 0:64], W("Wcohh"), hal[:, hp], start=False, stop=True)
                nc.tensor.matmul(nb[:, 64:65], W("W2ehh"), hrh[:], start=True, stop=False)
                nc.tensor.matmul(nb[:, 64:65], W("W2ehl"), hrh[:], start=False,
                                 stop=not full)
                if full:
                    nc.tensor.matmul(nb[:, 64:65], W("W2ehh"), hrl[:], start=False, stop=True)
                ne_h = stp.tile([128, 64], F16, name=f"ne_h{t}", tag="ne_h")
                nc.vector.tensor_scalar_add(ne_h[:], nb[:, 0:64], nb[:, 64:65])
                if full:
                    ne_l = stp.tile([128, 64], F16, name=f"ne_l{t}", tag="ne_l")
                    nc.vector.scalar_tensor_tensor(ne_l[:], nb[:, 0:64], nb[:, 64:65],
                                                   ne_h[:], ALU.add, ALU.subtract)

                # gates accumulate into the G bank chunks (start=False: the
                # G_pre contribution is already there)
                for g in range(4):
                    gsl = Gps[:, g * 768 + t * 64:g * 768 + (t + 1) * 64]
                    nc.tensor.matmul(gsl, Wg("Whh2h", g), hah[:, hp], start=False,
                                     stop=False, skip_group_check=True)
                    nc.tensor.matmul(gsl, Wg("Whh2l", g), hah[:, hp], start=False,
                                     stop=False, skip_group_check=True)
                    if full:
                        nc.tensor.matmul(gsl, Wg("Whh2h", g), hal[:, hp], start=False,
                                         stop=False, skip_group_check=True)
                    nc.tensor.matmul(gsl, Wg("Wihneh", g), ne_h[:], start=False,
                                     stop=False, skip_group_check=True)
                    nc.tensor.matmul(gsl, Wg("Wihnel", g), ne_h[:], start=False,
                                     stop=not full, skip_group_check=True)
                    if full:
                        nc.tensor.matmul(gsl, Wg("Wihneh", g), ne_l[:], start=False,
                                         stop=True, skip_group_check=True)

                Tt = stp.tile([128, 256], F32, name=f"T{t}", tag="T")
                nc.scalar.activation(Tt[:, 0:64], Gps[:, t * 64:t * 64 + 64],
                                     AF.Sigmoid, scale=2.0)
                nc.scalar.activation(Tt[:, 64:256], G3[:, 1:4, t * 64:t * 64 + 64],
                                     AF.Tanh)

                # carries for t+1 (skip at last step); bank6 double-buffered
                if t < T - 1:
                    sb = b6[:, 130 * ((t - 1) % 2):130 * ((t - 1) % 2) + 130]
                    nc.tensor.matmul(sb[:, 0:64], W("W3h"), S_h[:], start=True, stop=False)
                    nc.tensor.matmul(sb[:, 0:64], W("W3l"), S_h[:], start=False, stop=False)
                    if full:
                        nc.tensor.matmul(sb[:, 0:64], W("W3h"), S_l[:], start=False, stop=False)
                    nc.tensor.matmul(sb[:, 0:64], W("AW1hh"), hah[:, hp], start=False, stop=False)
                    nc.tensor.matmul(sb[:, 0:64], W("AW1hl"), hah[:, hp], start=False,
                                     stop=not full)
                    if full:
                        nc.tensor.matmul(sb[:, 0:64], W("AW1hh"), hal[:, hp], start=False, stop=True)
                    nc.tensor.matmul(sb[:, 128:129], W("W2hh"), hrh[:], start=True, stop=False)
                    nc.tensor.matmul(sb[:, 128:129], W("W2hl"), hrh[:], start=False,
                                     stop=not full)
                    if full:
                        nc.tensor.matmul(sb[:, 128:129], W("W2hh"), hrl[:], start=False, stop=True)
                    nc.tensor.matmul(sb[:, 64:128], W("Wssh"), se_h[:], start=True, stop=False)
                    nc.tensor.matmul(sb[:, 64:128], W("Wssl"), se_h[:], start=False, stop=False)
                    if full:
                        nc.tensor.matmul(sb[:, 64:128], W("Wssh"), se_l[:], start=False, stop=False)
                    nc.tensor.matmul(sb[:, 64:128], W("Wsh2h"), hah[:, hp], start=False, stop=False)
                    nc.tensor.matmul(sb[:, 64:128], W("Wsh2l"), hah[:, hp], start=False, stop=False)
                    if full:
                        nc.tensor.matmul(sb[:, 64:128], W("Wsh2h"), hal[:, hp], start=False, stop=False)
                    nc.tensor.matmul(sb[:, 64:128], W("Wstk"), tys[:], start=False, stop=True)
                    hv2 = stp.tile([128, 1], F32, name=f"hv2{t}", tag="hv2")
                    nc.scalar.activation(hv2[:], sb[:, 128:129], AF.Identity,
                                         bias=V("sbias"))
                    S_h = stp.tile([128, 64], F16, name=f"S_h{t}", tag="S_h")
                    nc.scalar.activation(S_h[:], sb[:, 0:64], AF.Identity,
                                         bias=hv2[:])
                    se_h = stp.tile([128, 64], F16, name=f"se_h{t}", tag="se_h")
                    nc.scalar.activation(se_h[:], sb[:, 64:128], AF.Identity,
                                         bias=V("bself"))
                    if t <= 7:
                        S_l = stp.tile([128, 64], F16, name=f"S_l{t}", tag="S_l")
                        nc.vector.scalar_tensor_tensor(S_l[:], sb[:, 0:64], hv2[:],
                                                       S_h[:], ALU.add, ALU.subtract)
                        se_l = stp.tile([128, 64], F16, name=f"se_l{t}", tag="se_l")
                        nc.vector.scalar_tensor_tensor(se_l[:], sb[:, 64:128],
                                                       V("bself"), se_h[:],
                                                       ALU.add, ALU.subtract)

                ctil, hrh, hrl = lstm_tail(t, Tt, ctil)
                if t == 1:
                    emit_G(0, 4, 8)
                    emit_G(1, 4, 8)
                elif t == 2:
                    emit_G(2, 4, 8)
                    emit_G(3, 4, 8)
                elif 3 <= t <= 6:
                    emit_G(t - 3, 8, 12)
                if t == 6:
                    emit_pred(0, 320)
                elif t == 10:
                    emit_pred(320, 640)

            emit_pred(640, 768)

    _split_excess_waits(nc)
    return dbg


# ---------------------------------------------------------------------------
def _make_in_maps(inputs):
    wt = _prep_weights(inputs)
    in_maps = []
    for b in range(B):
        ci = _prep_core_inputs(inputs, b)
        # fold the shared small weights into the per-core sm blob
        sm = ci["sm"]
        sm[0:6, _SM_WIN:_SM_WIN + 256] = wt["win"]
        sm[0:26, _SM_WCORR:_SM_WCORR + 512] = wt["wcorr"]
        sm[0:42, _SM_WSM:_SM_WSM + 384] = wt["wsm"]
        ci["wcg"] = wt["wcg"]
        ci["wmg"] = wt["wmg"]
        ci["wld"] = wt["wld"]
        ci["vec32"] = wt["vec32"]
        in_maps.append(ci)
    ones_fast = all(m.pop("ones_fast") for m in in_maps)
    if ones_fast:
        for m_ in in_maps:
            m_.pop("xag", None)
    return in_maps, ones_fast


def kernel(**inputs):
    in_maps, ones_fast = _make_in_maps(inputs)
    nc = bass.Bass("TRN2", target_bir_lowering=False, debug=False, num_devices=B)
    _build(nc, ones_fast)
    res = run_bass_kernel_spmd(nc, in_maps, list(range(B)))
    outs = []
    for b in range(B):
        o = res.results[b]["out"]                    # [128, 768] (O, t*64+a)
        outs.append(o.T.reshape(T, A, O).transpose(1, 0, 2))
    return np.stack(outs).astype(np.float32)         # [B, A, T, O]


if __name__ == "__main__":
    d = np.load("/root/problem/expected.npz")
    inputs = {k: d[k] for k in d.files if k != "expected"}
    out = kernel(**inputs)
    exp = d["expected"]
    err = np.abs(out - exp).max()
    print("absmax err:", err, "rel:", err / np.abs(exp).max())
